# revision 1
# baseline (speedup 1.0000x reference)
"""Trainium2 Bass kernel for a sparse (sliding-window) attention layer.

Reference computation (B=2, S=2048, D=2048, H=16 heads, window=256, fp32):
    qp = q @ Wq + bq ; kp = k @ Wk + bk ; vp = v @ Wv + bv
    per-head scores with mask (0 <= q_idx - k_idx <= 256), softmax, ctx
    out = merge_heads(ctx) @ Wo + bo
    returns (out, kp, vp)

Sharding: 8 cores = 2 (batch) x 4 (head groups of 4 heads / 512 dims).
Each core computes its batch's projections for its 512 output dims
(transposed layout for q/k so attention feeds straight into the PE),
the windowed attention for its 4 heads, and a partial out-projection
(rows of Wo owned by its heads).  Host sums the 4 partial outputs per
batch (the "out_proj all-reduce") and concatenates kp/vp slices.

Matmul operands are bf16 (fp32 accumulation in PSUM) -- 1 cycle/row on
the PE array; softmax statistics and all outputs stay fp32.  Set
KERNEL_MM_DT=f32 for a full-precision (4x slower) fallback.
"""

import os
import sys

import numpy as np

B = 2
S = 2048
D = 2048
GD = 512          # dims per core (4 heads x 128)
NH = 4            # heads per core
P = 128
WIN = 256         # sliding window
NDB = D // P      # 16 contraction blocks
SC = 512          # seq chunk for projections
NSC = S // SC     # 8
NSB = S // P      # 16 seq blocks
SCALE = 1.0 / np.sqrt(P)

_CACHE = {}
LAST_RESULTS = None


def _mm_dtype_name():
    return os.environ.get("KERNEL_MM_DT", "bf16")


def _build_nc():
    phases = os.environ.get("KERNEL_PHASES", "abc")
    sys.path.insert(0, "/opt/trn_rl_repo")
    import concourse.bass as bass  # noqa: F401
    import concourse.tile as tile
    from concourse import mybir, bacc
    from concourse.masks import make_identity
    from contextlib import ExitStack

    F32 = mybir.dt.float32
    CDT = mybir.dt.bfloat16 if _mm_dtype_name() == "bf16" else F32

    nc = bacc.Bacc("TRN2", target_bir_lowering=False, debug=False, num_devices=8)

    xq_T = nc.dram_tensor("xq_T", [D, S], CDT, kind="ExternalInput")
    xk_T = nc.dram_tensor("xk_T", [D, S], CDT, kind="ExternalInput")
    xv_T = nc.dram_tensor("xv_T", [D, S], CDT, kind="ExternalInput")
    wq = nc.dram_tensor("wq", [D, GD], CDT, kind="ExternalInput")
    wk = nc.dram_tensor("wk", [D, GD], CDT, kind="ExternalInput")
    wv = nc.dram_tensor("wv", [D, GD], CDT, kind="ExternalInput")
    wo = nc.dram_tensor("wo", [GD, D], CDT, kind="ExternalInput")
    bq2 = nc.dram_tensor("bq2", [GD], F32, kind="ExternalInput")
    bk2 = nc.dram_tensor("bk2", [GD], F32, kind="ExternalInput")
    bvb = nc.dram_tensor("bvb", [P, GD], F32, kind="ExternalInput")
    maskd = nc.dram_tensor("maskd", [P, 3 * P], F32, kind="ExternalInput")

    kpT_o = nc.dram_tensor("kpT", [GD, S], F32, kind="ExternalOutput")
    vp_o = nc.dram_tensor("vp", [S, GD], F32, kind="ExternalOutput")
    pout_o = nc.dram_tensor("pout", [S, D], F32, kind="ExternalOutput")

    # partition-major views of the DRAM operands
    xq_r = xq_T.ap().rearrange("(do p) s -> p do s", p=P)
    xk_r = xk_T.ap().rearrange("(do p) s -> p do s", p=P)
    xv_r = xv_T.ap().rearrange("(do p) s -> p do s", p=P)
    wq_r = wq.ap().rearrange("(do p) n -> p do n", p=P)
    wk_r = wk.ap().rearrange("(do p) n -> p do n", p=P)
    wv_r = wv.ap().rearrange("(do p) n -> p do n", p=P)
    wo_r = wo.ap().rearrange("(h p) n -> p h n", p=P)
    bq_r = bq2.ap().rearrange("(h p) -> p h", p=P)
    bk_r = bk2.ap().rearrange("(h p) -> p h", p=P)
    kpT_r = kpT_o.ap().rearrange("(h p) s -> h p s", p=P)

    AluOp = mybir.AluOpType
    ActFn = mybir.ActivationFunctionType

    with tile.TileContext(nc) as tc, ExitStack() as top:
        const = top.enter_context(tc.tile_pool(name="const", bufs=1))
        mask_sb = const.tile([P, 3 * P], F32, name="mask_sb")
        nc.sync.dma_start(mask_sb[:], maskd.ap())
        ident = const.tile([P, P], CDT, name="ident")
        make_identity(nc, ident[:])
        bq_sb = const.tile([P, NH], F32, name="bq_sb")
        nc.sync.dma_start(bq_sb[:], bq_r)
        bk_sb = const.tile([P, NH], F32, name="bk_sb")
        nc.sync.dma_start(bk_sb[:], bk_r)
        bvb_sb = const.tile([P, GD], F32, name="bvb_sb")
        nc.sync.dma_start(bvb_sb[:], bvb.ap())

        # long-lived activations
        persist1 = top.enter_context(tc.tile_pool(name="persist1", bufs=1))
        qpT = [persist1.tile([P, S], CDT, name=f"qpT{h}") for h in range(NH)]
        kpT = [persist1.tile([P, S], CDT, name=f"kpT{h}") for h in range(NH)]

        persist2 = top.enter_context(tc.tile_pool(name="persist2", bufs=1))
        vpB = [persist2.tile([P, GD], CDT, name=f"vpB{sb}") for sb in range(NSB)]
        persist3 = top.enter_context(tc.tile_pool(name="persist3", bufs=1))
        ctxT = [persist3.tile([P, S], CDT, name=f"ctxT{h}") for h in range(NH)]
        # B working pools opened early so attention work can overlap phase A
        wkp = top.enter_context(tc.tile_pool(name="wkp", bufs=3))
        psb = top.enter_context(tc.tile_pool(name="psb", bufs=2, space="PSUM"))
        pst = top.enter_context(tc.tile_pool(name="pst", bufs=2, space="PSUM"))
        psc = top.enter_context(tc.tile_pool(name="psc", bufs=2, space="PSUM"))

        # ---- phase A1: q/k projections (transposed): pT[d', s] ----
        with ExitStack() as a1:
          if "a" in phases:
              wpool = a1.enter_context(tc.tile_pool(name="wpool", bufs=1))
              wq_sb = wpool.tile([P, NDB, GD], CDT, name="wq_sb")
              nc.sync.dma_start(wq_sb[:], wq_r)
              wk_sb = wpool.tile([P, NDB, GD], CDT, name="wk_sb")
              nc.sync.dma_start(wk_sb[:], wk_r)
              xpool = a1.enter_context(tc.tile_pool(name="xpool", bufs=2))
              kfp = a1.enter_context(tc.tile_pool(name="kfp", bufs=3))
              pa = a1.enter_context(tc.tile_pool(name="pa", bufs=2, space="PSUM"))
              for x_r, w_sb, b_sb, is_k in (
                  (xq_r, wq_sb, bq_sb, False),
                  (xk_r, wk_sb, bk_sb, True),
              ):
                  for sc in range(NSC):
                      xt = xpool.tile([P, NDB, SC], CDT, tag="xa", name="xt")
                      nc.sync.dma_start(xt[:], x_r[:, :, sc * SC:(sc + 1) * SC])
                      for hb in range(NH):
                          ps = pa.tile([P, SC], F32, tag="pa", name="ps")
                          for db in range(NDB):
                              nc.tensor.matmul(
                                  ps[:],
                                  lhsT=w_sb[:, db, hb * P:(hb + 1) * P],
                                  rhs=xt[:, db, :],
                                  start=(db == 0),
                                  stop=(db == NDB - 1),
                              )
                          ssl = slice(sc * SC, (sc + 1) * SC)
                          if is_k:
                              kf = kfp.tile([P, SC], F32, tag="kf", name="kf")
                              nc.vector.tensor_scalar_add(
                                  kf[:], ps[:], b_sb[:, hb:hb + 1])
                              nc.scalar.copy(kpT[hb][:, ssl], kf[:])
                              nc.sync.dma_start(kpT_r[hb][:, ssl], kf[:])
                          else:
                              nc.vector.tensor_scalar_add(
                                  qpT[hb][:, ssl], ps[:], b_sb[:, hb:hb + 1])

        # ---- phase A2: v projection (natural layout): vp[s, d'] ----
        with ExitStack() as a2:
          if "v" in phases or "a" in phases:
              wpool2 = a2.enter_context(tc.tile_pool(name="wpool2", bufs=1))
              wv_sb = wpool2.tile([P, NDB, GD], CDT, name="wv_sb")
              nc.sync.dma_start(wv_sb[:], wv_r)
              xpool2 = a2.enter_context(tc.tile_pool(name="xpool2", bufs=2))
              vf_pool = a2.enter_context(tc.tile_pool(name="vf_pool", bufs=3))
              pa2 = a2.enter_context(tc.tile_pool(name="pa2", bufs=2, space="PSUM"))
              for sc in range(NSC):
                  xt2 = xpool2.tile([P, NDB, SC], CDT, tag="xa2", name="xt2")
                  nc.sync.dma_start(xt2[:], xv_r[:, :, sc * SC:(sc + 1) * SC])
                  for s2 in range(SC // P):
                      sb = sc * (SC // P) + s2
                      ps2 = pa2.tile([P, GD], F32, tag="pa2", name="ps2")
                      for db in range(NDB):
                          nc.tensor.matmul(
                              ps2[:],
                              lhsT=xt2[:, db, s2 * P:(s2 + 1) * P],
                              rhs=wv_sb[:, db, :],
                              start=(db == 0),
                              stop=(db == NDB - 1),
                          )
                      vf = vf_pool.tile([P, GD], F32, tag="vf", name="vf")
                      nc.vector.tensor_tensor(vf[:], ps2[:], bvb_sb[:], AluOp.add)
                      nc.scalar.copy(vpB[sb][:], vf[:])
                      nc.sync.dma_start(vp_o.ap()[sb * P:(sb + 1) * P, :], vf[:])

        # ---- phase B: windowed attention per (head, query block) ----
        with ExitStack() as bctx:
          if "b" in phases:
              for h in range(NH):
                  for qc in range(4):
                      kjbase = 0 if qc == 0 else 4 * qc - 2
                      nkj = 4 if qc == 0 else 6
                      pbuf = wkp.tile([P, 6, 512], CDT, tag="pbuf",
                                      name="pbuf", bufs=2)
                      nc.gpsimd.memset(pbuf[:], 0.0)
                      for t in range(4):
                          qb = qc * 4 + t
                          qs = qb * P
                          kj_lo = max(0, qs - WIN)
                          wdt = qs + P - kj_lo          # 128 / 256 / 384
                          moff = 3 * P - wdt
                          nblk = wdt // P
                          ps_s = psb.tile([P, 3 * P], F32, tag="ps_s", name="ps_s")
                          nc.tensor.matmul(
                              ps_s[:, :wdt],
                              lhsT=qpT[h][:, qs:qs + P],
                              rhs=kpT[h][:, kj_lo:kj_lo + wdt],
                              start=True,
                              stop=True,
                          )
                          nc.vector.tensor_tensor(
                              ps_s[:, :wdt], ps_s[:, :wdt],
                              mask_sb[:, moff:moff + wdt], AluOp.add)
                          exps = wkp.tile([P, 3 * P], CDT, tag="exps", name="exps")
                          rsum = wkp.tile([P, 1], F32, tag="rsum", name="rsum")
                          nc.scalar.activation(exps[:, :wdt], ps_s[:, :wdt],
                                               ActFn.Exp, scale=float(SCALE),
                                               accum_out=rsum[:])
                          rinv = wkp.tile([P, 1], F32, tag="rinv", name="rinv")
                          nc.vector.reciprocal(rinv[:], rsum[:])
                          probB = wkp.tile([P, 3 * P], CDT, tag="probB", name="probB")
                          nc.vector.tensor_scalar_mul(probB[:, :wdt],
                                                      exps[:, :wdt], rinv[:])
                          for j in range(nblk):
                              ps_t = pst.tile([P, P], CDT, tag="ps_t", name="ps_t",
                                              bufs=2)
                              nc.tensor.transpose(ps_t[:],
                                                  probB[:, j * P:(j + 1) * P],
                                                  ident[:])
                              rel = (kj_lo // P + j) - kjbase
                              nc.vector.tensor_copy(
                                  pbuf[:, rel, t * P:(t + 1) * P], ps_t[:])
                      ps_pv = psc.tile([P, 512], F32, tag="ps_pv", name="ps_pv")
                      for j2 in range(nkj):
                          nc.tensor.matmul(
                              ps_pv[:],
                              lhsT=vpB[kjbase + j2][:, h * P:(h + 1) * P],
                              rhs=pbuf[:, j2, :],
                              start=(j2 == 0),
                              stop=(j2 == nkj - 1),
                          )
                      nc.scalar.copy(ctxT[h][:, qc * 512:(qc + 1) * 512], ps_pv[:])

        # ---- phase C: partial out-projection pout = ctx @ Wo_g ----
        with ExitStack() as c:
          if "c" in phases:
              cpool = c.enter_context(tc.tile_pool(name="cpool", bufs=2))
              wopool = c.enter_context(tc.tile_pool(name="wopool", bufs=1))
              wo_sb = wopool.tile([P, NH, D], CDT, name="wo_sb")
              nc.sync.dma_start(wo_sb[:], wo_r)
              psC = c.enter_context(tc.tile_pool(name="psC", bufs=2, space="PSUM"))
              for sb in range(NSB):
                  po = cpool.tile([P, D], F32, tag="po", name="po")
                  for ec in range(D // 512):
                      psq = psC.tile([P, 512], F32, tag="psq", name="psq")
                      for h in range(NH):
                          nc.tensor.matmul(
                              psq[:],
                              lhsT=ctxT[h][:, sb * P:(sb + 1) * P],
                              rhs=wo_sb[:, h, ec * 512:(ec + 1) * 512],
                              start=(h == 0),
                              stop=(h == NH - 1),
                          )
                      nc.vector.tensor_copy(po[:, ec * 512:(ec + 1) * 512], psq[:])
                  nc.sync.dma_start(pout_o.ap()[sb * P:(sb + 1) * P, :], po[:])

    nc.compile()
    return nc


def _band_mask():
    i = np.arange(P)[:, None]
    j = np.arange(3 * P)[None, :]
    return np.where((j >= i) & (j <= i + WIN), 0.0, -1e6).astype(np.float32)


def kernel(q, k, v, Wq, bq, Wk, bk, Wv, bv, Wo, bo):
    global LAST_RESULTS
    q = np.asarray(q, np.float32)
    k = np.asarray(k, np.float32)
    v = np.asarray(v, np.float32)
    Wq = np.asarray(Wq, np.float32)
    Wk = np.asarray(Wk, np.float32)
    Wv = np.asarray(Wv, np.float32)
    Wo = np.asarray(Wo, np.float32)
    bq = np.asarray(bq, np.float32)
    bk = np.asarray(bk, np.float32)
    bv = np.asarray(bv, np.float32)
    bo = np.asarray(bo, np.float32)

    if "nc" not in _CACHE:
        _CACHE["nc"] = _build_nc()
    nc = _CACHE["nc"]
    from concourse.bass_utils import run_bass_kernel_spmd

    if _mm_dtype_name() == "bf16":
        import ml_dtypes

        cdt = ml_dtypes.bfloat16
    else:
        cdt = np.float32

    mask = _band_mask()
    xT = {}
    for b in range(B):
        xT[("q", b)] = np.ascontiguousarray(q[b].T).astype(cdt)
        xT[("k", b)] = np.ascontiguousarray(k[b].T).astype(cdt)
        xT[("v", b)] = np.ascontiguousarray(v[b].T).astype(cdt)

    in_maps = []
    for core in range(8):
        b, g = divmod(core, 4)
        sl = slice(g * GD, (g + 1) * GD)
        in_maps.append({
            "xq_T": xT[("q", b)],
            "xk_T": xT[("k", b)],
            "xv_T": xT[("v", b)],
            "wq": np.ascontiguousarray(Wq[:, sl]).astype(cdt),
            "wk": np.ascontiguousarray(Wk[:, sl]).astype(cdt),
            "wv": np.ascontiguousarray(Wv[:, sl]).astype(cdt),
            "wo": np.ascontiguousarray(Wo[sl, :]).astype(cdt),
            "bq2": np.ascontiguousarray(bq[sl]),
            "bk2": np.ascontiguousarray(bk[sl]),
            "bvb": np.ascontiguousarray(np.broadcast_to(bv[sl], (P, GD))),
            "maskd": mask,
        })

    trace = os.environ.get("KERNEL_TRACE", "0") == "1"
    res = run_bass_kernel_spmd(nc, in_maps, core_ids=list(range(8)), trace=trace)
    LAST_RESULTS = res

    out = np.zeros((B, S, D), np.float64)
    kp = np.empty((B, S, D), np.float32)
    vp = np.empty((B, S, D), np.float32)
    for core in range(8):
        b, g = divmod(core, 4)
        sl = slice(g * GD, (g + 1) * GD)
        r = res.results[core]
        kp[b][:, sl] = r["kpT"].T
        vp[b][:, sl] = r["vp"]
        out[b] += r["pout"].astype(np.float64)
    out = (out + bo.astype(np.float64)).astype(np.float32)
    return out, kp, vp



# revision 2
# speedup vs baseline: 1.2751x; 1.2751x over previous
"""Trainium2 Bass kernel for a sparse (sliding-window) attention layer.

Reference computation (B=2, S=2048, D=2048, H=16 heads, window=256, fp32):
    qp = q @ Wq + bq ; kp = k @ Wk + bk ; vp = v @ Wv + bv
    per-head scores with mask (0 <= q_idx - k_idx <= 256), softmax, ctx
    out = merge_heads(ctx) @ Wo + bo
    returns (out, kp, vp)

Sharding: 8 cores = 2 (batch) x 4 (head groups of 4 heads / 512 dims).
Each core computes its batch's projections for its 512 output dims,
windowed attention for its 4 heads, and a partial out-projection (rows
of Wo owned by its heads).  Host sums the 4 bf16 partial outputs per
batch and concatenates kp/vp slices.

Attention is computed in transposed form: S^T[k, q] = kpT^T @ qpT per
key block (one N=384 matmul covering the 3 query blocks in this key
block's window), exp on the scalar engine, triangular masking with
gpsimd affine_select (zero-fill), and PV as ctx[q, dh] with lhsT =
exp^T blocks; a ones-column appended to v yields the softmax row sums
in the same matmul.  This removes all probability transposes of the
previous design; only one [128,128] ctx transpose per query block
remains (to feed the out-projection).

Matmul operands are bf16 (fp32 PSUM accumulation); outputs are written
bf16 and widened on host (tolerance 2e-2 >> bf16 rounding).
"""

import os

import numpy as np

B = 2
S = 2048
D = 2048
GD = 512          # dims per core (4 heads x 128)
NH = 4            # heads per core
P = 128
WIN = 256         # sliding window
NDB = D // P      # 16 contraction blocks
SC = 512          # seq chunk for projections
NSC = S // SC     # 4
NSB = S // P      # 16 seq blocks
NRING = 8         # exp ring slots per head
SCALE = 1.0 / np.sqrt(P)

_CACHE = {}
LAST_RESULTS = None

# attention blocks that become computable after q-projection chunk c:
# scores(kb) needs qpT blocks kb..kb+2 ; PV(qb) needs exp(kb=qb) => qb+2
_KB_BATCH = [range(0, 2), range(2, 6), range(6, 10), range(10, 16)]
_QB_BATCH = [range(0, 2), range(2, 6), range(6, 10), range(10, 16)]


def _build_nc():
    import sys
    sys.path.insert(0, "/opt/trn_rl_repo")
    import concourse.bass as bass  # noqa: F401
    import concourse.tile as tile
    from concourse import mybir, bacc
    from concourse.masks import make_identity
    from contextlib import ExitStack

    F32 = mybir.dt.float32
    CDT = mybir.dt.bfloat16
    AluOp = mybir.AluOpType
    ActFn = mybir.ActivationFunctionType

    nc = bacc.Bacc("TRN2", target_bir_lowering=False, debug=False, num_devices=8)

    # host-packed inputs: x*_c[(c p), (do s)] = x^T[do*128+p, c*512+s]
    xq_c = nc.dram_tensor("xq_c", [NSC * P, NDB * SC], CDT, kind="ExternalInput")
    xk_c = nc.dram_tensor("xk_c", [NSC * P, NDB * SC], CDT, kind="ExternalInput")
    xv_c = nc.dram_tensor("xv_c", [NSC * P, NDB * SC], CDT, kind="ExternalInput")
    # w*_c[p, (do n)] = W[do*128+p, g*512+n]
    wq_c = nc.dram_tensor("wq_c", [P, NDB * GD], CDT, kind="ExternalInput")
    wk_c = nc.dram_tensor("wk_c", [P, NDB * GD], CDT, kind="ExternalInput")
    wv_c = nc.dram_tensor("wv_c", [P, NDB * GD], CDT, kind="ExternalInput")
    # wo_c[p, (h n)] = Wo[g*512 + h*128 + p, n]
    wo_c = nc.dram_tensor("wo_c", [P, NH * D], CDT, kind="ExternalInput")
    bq_c = nc.dram_tensor("bq_c", [P, NH], F32, kind="ExternalInput")
    bk_c = nc.dram_tensor("bk_c", [P, NH], F32, kind="ExternalInput")
    bvb = nc.dram_tensor("bvb", [P, GD], F32, kind="ExternalInput")

    kp_o = nc.dram_tensor("kp", [GD, S], CDT, kind="ExternalOutput")
    vp_o = nc.dram_tensor("vp", [S, GD], CDT, kind="ExternalOutput")
    pout_o = nc.dram_tensor("pout", [S, D], CDT, kind="ExternalOutput")

    xq_r = xq_c.ap().rearrange("(c p) m -> c p m", p=P)
    xk_r = xk_c.ap().rearrange("(c p) m -> c p m", p=P)
    xv_r = xv_c.ap().rearrange("(c p) m -> c p m", p=P)
    kp_r = kp_o.ap().rearrange("(hb p) s -> hb p s", p=P)
    vp_r = vp_o.ap().rearrange("(sb p) (h d) -> sb p h d", p=P, h=NH)

    with tile.TileContext(nc) as tc, ExitStack() as top:
        const = top.enter_context(tc.tile_pool(name="const", bufs=1))
        ident = const.tile([P, P], CDT, name="ident")
        make_identity(nc, ident[:])
        bq_sb = const.tile([P, NH], F32, name="bq_sb")
        nc.sync.dma_start(bq_sb[:], bq_c.ap())
        bk_sb = const.tile([P, NH], F32, name="bk_sb")
        nc.sync.dma_start(bk_sb[:], bk_c.ap())
        bvb_sb = const.tile([P, NH, P], F32, name="bvb_sb")
        nc.sync.dma_start(bvb_sb[:], bvb.ap())

        # long-lived activations
        persist1 = top.enter_context(tc.tile_pool(name="persist1", bufs=1))
        qpT = [persist1.tile([P, S], CDT, name=f"qpT{h}") for h in range(NH)]
        kpT = [persist1.tile([P, S], CDT, name=f"kpT{h}") for h in range(NH)]
        persist2 = top.enter_context(tc.tile_pool(name="persist2", bufs=1))
        vpB = [persist2.tile([P, NH, P + 1], CDT, name=f"vpB{sb}")
               for sb in range(NSB)]
        persist3 = top.enter_context(tc.tile_pool(name="persist3", bufs=1))
        ctxT = [persist3.tile([P, S], CDT, name=f"ctxT{h}") for h in range(NH)]
        expP = top.enter_context(tc.tile_pool(name="expP", bufs=1))
        expT = [expP.tile([P, NRING, 3 * P], CDT, name=f"expT{h}")
                for h in range(NH)]
        for sb in range(NSB):
            nc.gpsimd.memset(vpB[sb][:, :, P:P + 1], 1.0)

        # out-proj weights loaded early (DMA overlaps phase A)
        wopool = top.enter_context(tc.tile_pool(name="wopool", bufs=1))
        wo_sb = wopool.tile([P, NH, D], CDT, name="wo_sb")

        # attention working pools (SBUF)
        awk = top.enter_context(tc.tile_pool(name="awk", bufs=3))

        with ExitStack() as actx:
            wpool = actx.enter_context(tc.tile_pool(name="wpool", bufs=1))
            wk_sb = wpool.tile([P, NDB, GD], CDT, name="wk_sb")
            for i in range(4):  # split so first matmul starts ~1.5us in
                nc.sync.dma_start(wk_sb[:, 4 * i:4 * i + 4, :],
                                  wk_c.ap()[:, 4 * i * GD:(4 * i + 4) * GD])
            wv_sb = wpool.tile([P, NDB, GD], CDT, name="wv_sb")
            nc.sync.dma_start(wv_sb[:], wv_c.ap())
            wq_sb = wpool.tile([P, NDB, GD], CDT, name="wq_sb")
            nc.sync.dma_start(wq_sb[:], wq_c.ap())
            nc.sync.dma_start(wo_sb[:], wo_c.ap())

            xpool = actx.enter_context(tc.tile_pool(name="xpool", bufs=2))
            pa = actx.enter_context(tc.tile_pool(name="pa", bufs=2,
                                                 space="PSUM"))

            # ---- k projection (transposed): kpT[d', s] ----
            for sc in range(NSC):
                xt = xpool.tile([P, NDB, SC], CDT, tag="x", name="xt")
                if sc == 0:
                    for i in range(4):
                        nc.sync.dma_start(
                            xt[:, 4 * i:4 * i + 4, :],
                            xk_r[0][:, 4 * i * SC:(4 * i + 4) * SC])
                else:
                    nc.sync.dma_start(xt[:], xk_r[sc])
                ssl = slice(sc * SC, (sc + 1) * SC)
                for hb in range(NH):
                    ps = pa.tile([P, SC], F32, tag="pa", name="ps")
                    for db in range(NDB):
                        nc.tensor.matmul(
                            ps[:],
                            lhsT=wk_sb[:, db, hb * P:(hb + 1) * P],
                            rhs=xt[:, db, :],
                            start=(db == 0),
                            stop=(db == NDB - 1),
                        )
                    nc.vector.tensor_scalar_add(kpT[hb][:, ssl], ps[:],
                                                bk_sb[:, hb:hb + 1])
                    nc.sync.dma_start(kp_r[hb][:, ssl], kpT[hb][:, ssl])

            # ---- v projection (natural layout + ones col): vpB[s, h, d'] ----
            with ExitStack() as vctx:
                pav = vctx.enter_context(tc.tile_pool(name="pav", bufs=2,
                                                      space="PSUM"))
                for sc in range(NSC):
                    xt = xpool.tile([P, NDB, SC], CDT, tag="x", name="xt")
                    nc.sync.dma_start(xt[:], xv_r[sc])
                    for s2 in range(SC // P):
                        sb = sc * (SC // P) + s2
                        ps2 = pav.tile([P, NH, P], F32, tag="pav", name="ps2")
                        for db in range(NDB):
                            nc.tensor.matmul(
                                ps2[:],
                                lhsT=xt[:, db, s2 * P:(s2 + 1) * P],
                                rhs=wv_sb[:, db, :],
                                start=(db == 0),
                                stop=(db == NDB - 1),
                            )
                        nc.vector.tensor_tensor(vpB[sb][:, :, 0:P], ps2[:],
                                                bvb_sb[:], AluOp.add)
                        nc.sync.dma_start(vp_r[sb], vpB[sb][:, :, 0:P])

            # attention PSUM pools (opened after pav closes: stay <= 8 banks)
            scps = actx.enter_context(tc.tile_pool(name="scps", bufs=2,
                                                   space="PSUM"))
            pvps = actx.enter_context(tc.tile_pool(name="pvps", bufs=2,
                                                   space="PSUM"))
            trps = actx.enter_context(tc.tile_pool(name="trps", bufs=2,
                                                   space="PSUM"))

            # ---- q projection interleaved with windowed attention ----
            for sc in range(NSC):
                xt = xpool.tile([P, NDB, SC], CDT, tag="x", name="xt")
                nc.sync.dma_start(xt[:], xq_r[sc])
                ssl = slice(sc * SC, (sc + 1) * SC)
                for hb in range(NH):
                    ps = pa.tile([P, SC], F32, tag="pa", name="ps")
                    for db in range(NDB):
                        nc.tensor.matmul(
                            ps[:],
                            lhsT=wq_sb[:, db, hb * P:(hb + 1) * P],
                            rhs=xt[:, db, :],
                            start=(db == 0),
                            stop=(db == NDB - 1),
                        )
                    nc.vector.tensor_scalar_add(qpT[hb][:, ssl], ps[:],
                                                bq_sb[:, hb:hb + 1])

                # scores + exp + masks for key blocks unlocked by this chunk
                for h in range(NH):
                    for kb in _KB_BATCH[sc]:
                        nj = min(3, NSB - kb)
                        kr = kb % NRING
                        scp = scps.tile([P, 3 * P], F32, tag="sc", name="scp")
                        nc.tensor.matmul(
                            scp[:, :nj * P],
                            lhsT=kpT[h][:, kb * P:(kb + 1) * P],
                            rhs=qpT[h][:, kb * P:(kb + nj) * P],
                            start=True,
                            stop=True,
                        )
                        nc.scalar.activation(expT[h][:, kr, 0:nj * P],
                                             scp[:, :nj * P], ActFn.Exp,
                                             scale=float(SCALE))
                        # diag block: keep q >= k  (iota = qf - kp >= 0)
                        nc.gpsimd.affine_select(
                            out=expT[h][:, kr, 0:P],
                            in_=expT[h][:, kr, 0:P],
                            pattern=[[1, P]],
                            channel_multiplier=-1,
                            base=0,
                            compare_op=AluOp.is_ge,
                            fill=0.0,
                        )
                        if nj == 3:
                            # far block: keep q <= k  (iota = kp - qf >= 0)
                            nc.gpsimd.affine_select(
                                out=expT[h][:, kr, 2 * P:3 * P],
                                in_=expT[h][:, kr, 2 * P:3 * P],
                                pattern=[[-1, P]],
                                channel_multiplier=1,
                                base=0,
                                compare_op=AluOp.is_ge,
                                fill=0.0,
                            )

                # PV + normalize + transpose for query blocks now complete
                for h in range(NH):
                    for qb in _QB_BATCH[sc]:
                        kb0 = max(0, qb - 2)
                        pv = pvps.tile([P, P + 1], F32, tag="pv", name="pv")
                        for kb in range(kb0, qb + 1):
                            rel = qb - kb
                            nc.tensor.matmul(
                                pv[:],
                                lhsT=expT[h][:, kb % NRING,
                                             rel * P:(rel + 1) * P],
                                rhs=vpB[kb][:, h, :],
                                start=(kb == kb0),
                                stop=(kb == qb),
                            )
                        rinv = awk.tile([P, 1], F32, tag="rinv", name="rinv")
                        nc.vector.reciprocal(rinv[:], pv[:, P:P + 1])
                        csb = awk.tile([P, P], CDT, tag="csb", name="csb")
                        nc.vector.tensor_scalar_mul(csb[:], pv[:, 0:P],
                                                    rinv[:])
                        tp = trps.tile([P, P], CDT, tag="tp", name="tp")
                        nc.tensor.transpose(tp[:], csb[:], ident[:])
                        nc.scalar.copy(ctxT[h][:, qb * P:(qb + 1) * P], tp[:])

        # ---- phase C: partial out-projection pout = ctx @ Wo_g ----
        with ExitStack() as cctx:
            cpool = cctx.enter_context(tc.tile_pool(name="cpool", bufs=2))
            psC = cctx.enter_context(tc.tile_pool(name="psC", bufs=2,
                                                  space="PSUM"))
            for sb in range(NSB):
                po = cpool.tile([P, D], CDT, tag="po", name="po")
                for ec in range(D // 512):
                    psq = psC.tile([P, 512], F32, tag="psq", name="psq")
                    for h in range(NH):
                        nc.tensor.matmul(
                            psq[:],
                            lhsT=ctxT[h][:, sb * P:(sb + 1) * P],
                            rhs=wo_sb[:, h, ec * 512:(ec + 1) * 512],
                            start=(h == 0),
                            stop=(h == NH - 1),
                        )
                    nc.scalar.copy(po[:, ec * 512:(ec + 1) * 512], psq[:])
                nc.sync.dma_start(pout_o.ap()[sb * P:(sb + 1) * P, :], po[:])

    nc.compile()
    return nc


def kernel(q, k, v, Wq, bq, Wk, bk, Wv, bv, Wo, bo):
    global LAST_RESULTS
    import ml_dtypes

    cdt = ml_dtypes.bfloat16
    q = np.asarray(q, np.float32)
    k = np.asarray(k, np.float32)
    v = np.asarray(v, np.float32)
    Wq = np.asarray(Wq, np.float32)
    Wk = np.asarray(Wk, np.float32)
    Wv = np.asarray(Wv, np.float32)
    Wo = np.asarray(Wo, np.float32)
    bq = np.asarray(bq, np.float32)
    bk = np.asarray(bk, np.float32)
    bv = np.asarray(bv, np.float32)
    bo = np.asarray(bo, np.float32)

    if "nc" not in _CACHE:
        _CACHE["nc"] = _build_nc()
    nc = _CACHE["nc"]
    from concourse.bass_utils import run_bass_kernel_spmd

    def pack_x(x):  # [S, D] -> [(c p), (do s)] with x^T chunked along seq
        a = x.T.reshape(NDB, P, NSC, SC)
        return np.ascontiguousarray(
            a.transpose(2, 1, 0, 3).reshape(NSC * P, NDB * SC)).astype(cdt)

    def pack_w(W, gsl):  # [D, D] cols gsl -> [p, (do n)]
        a = W[:, gsl].reshape(NDB, P, GD)
        return np.ascontiguousarray(
            a.transpose(1, 0, 2).reshape(P, NDB * GD)).astype(cdt)

    def pack_wo(W, gsl):  # rows gsl -> [p, (h n)]
        a = W[gsl, :].reshape(NH, P, D)
        return np.ascontiguousarray(
            a.transpose(1, 0, 2).reshape(P, NH * D)).astype(cdt)

    xs = {}
    for b in range(B):
        xs[("q", b)] = pack_x(q[b])
        xs[("k", b)] = pack_x(k[b])
        xs[("v", b)] = pack_x(v[b])

    in_maps = []
    for core in range(8):
        b, g = divmod(core, 4)
        gsl = slice(g * GD, (g + 1) * GD)
        in_maps.append({
            "xq_c": xs[("q", b)],
            "xk_c": xs[("k", b)],
            "xv_c": xs[("v", b)],
            "wq_c": pack_w(Wq, gsl),
            "wk_c": pack_w(Wk, gsl),
            "wv_c": pack_w(Wv, gsl),
            "wo_c": pack_wo(Wo, gsl),
            "bq_c": np.ascontiguousarray(bq[gsl].reshape(NH, P).T),
            "bk_c": np.ascontiguousarray(bk[gsl].reshape(NH, P).T),
            "bvb": np.ascontiguousarray(
                np.broadcast_to(bv[gsl], (P, GD))).astype(np.float32),
        })

    trace = os.environ.get("KERNEL_TRACE", "0") == "1"
    res = run_bass_kernel_spmd(nc, in_maps, core_ids=list(range(8)),
                               trace=trace)
    LAST_RESULTS = res

    out = np.zeros((B, S, D), np.float64)
    kp = np.empty((B, S, D), np.float32)
    vp = np.empty((B, S, D), np.float32)
    for core in range(8):
        b, g = divmod(core, 4)
        gsl = slice(g * GD, (g + 1) * GD)
        r = res.results[core]
        kp[b][:, gsl] = r["kp"].astype(np.float32).T
        vp[b][:, gsl] = r["vp"].astype(np.float32)
        out[b] += r["pout"].astype(np.float64)
    out = (out + bo.astype(np.float64)).astype(np.float32)
    return out, kp, vp


# revision 4
# speedup vs baseline: 1.3265x; 1.0403x over previous
"""Trainium2 Bass kernel for a sparse (sliding-window) attention layer.

Reference computation (B=2, S=2048, D=2048, H=16 heads, window=256, fp32):
    qp = q @ Wq + bq ; kp = k @ Wk + bk ; vp = v @ Wv + bv
    per-head scores with mask (0 <= q_idx - k_idx <= 256), softmax, ctx
    out = merge_heads(ctx) @ Wo + bo
    returns (out, kp, vp)

Sharding: 8 cores = 2 (batch) x 4 (head groups of 4 heads / 512 dims).
Each core computes its batch's projections for its 512 output dims,
windowed attention for its 4 heads, and a partial out-projection (rows
of Wo owned by its heads).  Host sums the 4 bf16 partial outputs per
batch and concatenates kp/vp slices.

Attention is computed in transposed form: S^T[k, q] = kpT^T @ qpT per
key block (one N=384 matmul covering the 3 query blocks in this key
block's window), exp on the scalar engine, triangular masking with
gpsimd affine_select (zero-fill), and PV as ctx[q, dh] with lhsT =
exp^T blocks; a ones-column appended to v yields the softmax row sums
in the same matmul.  This removes all probability transposes of the
previous design; only one [128,128] ctx transpose per query block
remains (to feed the out-projection).

Matmul operands are bf16 (fp32 PSUM accumulation); outputs are written
bf16 and widened on host (tolerance 2e-2 >> bf16 rounding).
"""

import os

import numpy as np

B = 2
S = 2048
D = 2048
GD = 512          # dims per core (4 heads x 128)
NH = 4            # heads per core
P = 128
WIN = 256         # sliding window
NDB = D // P      # 16 contraction blocks
SC = 512          # seq chunk for projections
NSC = S // SC     # 4
NSB = S // P      # 16 seq blocks
NRING = 8         # exp ring slots per head
SCALE = 1.0 / np.sqrt(P)

_CACHE = {}
LAST_RESULTS = None

# attention blocks that become computable after q-projection chunk c:
# scores(kb) needs qpT blocks kb..kb+2 ; PV(qb) needs exp(kb=qb) => qb+2
_KB_BATCH = [range(0, 2), range(2, 6), range(6, 10), range(10, 16)]
_QB_BATCH = [range(0, 2), range(2, 6), range(6, 10), range(10, 16)]


def _build_nc():
    import sys
    sys.path.insert(0, "/opt/trn_rl_repo")
    import concourse.bass as bass  # noqa: F401
    import concourse.tile as tile
    from concourse import mybir, bacc
    from concourse.masks import make_identity
    from contextlib import ExitStack

    F32 = mybir.dt.float32
    CDT = mybir.dt.bfloat16
    AluOp = mybir.AluOpType
    ActFn = mybir.ActivationFunctionType

    nc = bacc.Bacc("TRN2", target_bir_lowering=False, debug=False, num_devices=8)

    # host-packed inputs: x*_c[(c p), (do s)] = x^T[do*128+p, c*512+s]
    xq_c = nc.dram_tensor("xq_c", [NSC * P, NDB * SC], CDT, kind="ExternalInput")
    xk_c = nc.dram_tensor("xk_c", [NSC * P, NDB * SC], CDT, kind="ExternalInput")
    xv_c = nc.dram_tensor("xv_c", [NSC * P, NDB * SC], CDT, kind="ExternalInput")
    # w*_c[p, (do n)] = W[do*128+p, g*512+n]
    wq_c = nc.dram_tensor("wq_c", [P, NDB * GD], CDT, kind="ExternalInput")
    wk_c = nc.dram_tensor("wk_c", [P, NDB * GD], CDT, kind="ExternalInput")
    wv_c = nc.dram_tensor("wv_c", [P, NDB * GD], CDT, kind="ExternalInput")
    # wo_c[p, (h n)] = Wo[g*512 + h*128 + p, n]
    wo_c = nc.dram_tensor("wo_c", [P, NH * D], CDT, kind="ExternalInput")
    bq_c = nc.dram_tensor("bq_c", [P, NH], F32, kind="ExternalInput")
    bk_c = nc.dram_tensor("bk_c", [P, NH], F32, kind="ExternalInput")
    bvb = nc.dram_tensor("bvb", [P, GD], F32, kind="ExternalInput")

    kp_o = nc.dram_tensor("kp", [GD, S], CDT, kind="ExternalOutput")
    vp_o = nc.dram_tensor("vp", [S, GD], CDT, kind="ExternalOutput")
    pout_o = nc.dram_tensor("pout", [S, D], CDT, kind="ExternalOutput")

    xq_r = xq_c.ap().rearrange("(c p) m -> c p m", p=P)
    xk_r = xk_c.ap().rearrange("(c p) m -> c p m", p=P)
    xv_r = xv_c.ap().rearrange("(c p) m -> c p m", p=P)
    kp_r = kp_o.ap().rearrange("(hb p) s -> hb p s", p=P)
    vp_r = vp_o.ap().rearrange("(sb p) (h d) -> sb p h d", p=P, h=NH)

    with tile.TileContext(nc) as tc, ExitStack() as top:
        const = top.enter_context(tc.tile_pool(name="const", bufs=1))
        ident = const.tile([P, P], CDT, name="ident")
        make_identity(nc, ident[:])
        bq_sb = const.tile([P, NH], F32, name="bq_sb")
        nc.sync.dma_start(bq_sb[:], bq_c.ap())
        bk_sb = const.tile([P, NH], F32, name="bk_sb")
        nc.sync.dma_start(bk_sb[:], bk_c.ap())
        bvb_sb = const.tile([P, NH, P], F32, name="bvb_sb")
        nc.sync.dma_start(bvb_sb[:], bvb.ap())

        # long-lived activations
        persist1 = top.enter_context(tc.tile_pool(name="persist1", bufs=1))
        qpT = [persist1.tile([P, S], CDT, name=f"qpT{h}") for h in range(NH)]
        kpT = [persist1.tile([P, S], CDT, name=f"kpT{h}") for h in range(NH)]
        persist2 = top.enter_context(tc.tile_pool(name="persist2", bufs=1))
        vpB = [persist2.tile([P, NH, P + 1], CDT, name=f"vpB{sb}")
               for sb in range(NSB)]
        persist3 = top.enter_context(tc.tile_pool(name="persist3", bufs=1))
        ctxT = [persist3.tile([P, S], CDT, name=f"ctxT{h}") for h in range(NH)]
        expP = top.enter_context(tc.tile_pool(name="expP", bufs=1))
        expT = [expP.tile([P, NRING, 3 * P], CDT, name=f"expT{h}")
                for h in range(NH)]
        for sb in range(NSB):
            nc.gpsimd.memset(vpB[sb][:, :, P:P + 1], 1.0)

        # out-proj weights loaded early (DMA overlaps phase A)
        wopool = top.enter_context(tc.tile_pool(name="wopool", bufs=1))
        wo_sb = wopool.tile([P, NH, D], CDT, name="wo_sb")

        # attention working pools (SBUF)
        awk = top.enter_context(tc.tile_pool(name="awk", bufs=3))

        with ExitStack() as actx:
            wpool = actx.enter_context(tc.tile_pool(name="wpool", bufs=1))
            xpool = actx.enter_context(tc.tile_pool(name="xpool", bufs=2))
            pa = actx.enter_context(tc.tile_pool(name="pa", bufs=2,
                                                 space="PSUM"))

            # warmup: interleave first x chunk and wk in small pieces so the
            # first matmul starts ~2us in; big prefetches are emitted later
            # (between k chunks) so they cannot queue ahead of these.
            wk_sb = wpool.tile([P, NDB, GD], CDT, name="wk_sb")
            xt0 = xpool.tile([P, NDB, SC], CDT, tag="x", name="xt")
            for i in range(4):
                nc.sync.dma_start(xt0[:, 4 * i:4 * i + 4, :],
                                  xk_r[0][:, 4 * i * SC:(4 * i + 4) * SC])
                nc.sync.dma_start(wk_sb[:, 4 * i:4 * i + 4, :],
                                  wk_c.ap()[:, 4 * i * GD:(4 * i + 4) * GD])
            wv_sb = wpool.tile([P, NDB, GD], CDT, name="wv_sb")
            wq_sb = wpool.tile([P, NDB, GD], CDT, name="wq_sb")

            # ---- k projection (transposed): kpT[d', s] ----
            for sc in range(NSC):
                if sc == 0:
                    xt = xt0
                else:
                    xt = xpool.tile([P, NDB, SC], CDT, tag="x", name="xt")
                    nc.sync.dma_start(xt[:], xk_r[sc])
                if sc == 1:
                    nc.sync.dma_start(wv_sb[:], wv_c.ap())
                elif sc == 2:
                    nc.sync.dma_start(wq_sb[:], wq_c.ap())
                elif sc == 3:
                    nc.sync.dma_start(wo_sb[:], wo_c.ap())
                ssl = slice(sc * SC, (sc + 1) * SC)
                for hb in range(NH):
                    ps = pa.tile([P, SC], F32, tag="pa", name="ps")
                    for db in range(NDB):
                        nc.tensor.matmul(
                            ps[:],
                            lhsT=wk_sb[:, db, hb * P:(hb + 1) * P],
                            rhs=xt[:, db, :],
                            start=(db == 0),
                            stop=(db == NDB - 1),
                        )
                    nc.vector.tensor_scalar_add(kpT[hb][:, ssl], ps[:],
                                                bk_sb[:, hb:hb + 1])
                    nc.sync.dma_start(kp_r[hb][:, ssl], kpT[hb][:, ssl])

            # ---- v projection (natural layout + ones col): vpB[s, h, d'] ----
            with ExitStack() as vctx:
                pav = vctx.enter_context(tc.tile_pool(name="pav", bufs=2,
                                                      space="PSUM"))
                for sc in range(NSC):
                    xt = xpool.tile([P, NDB, SC], CDT, tag="x", name="xt")
                    nc.sync.dma_start(xt[:], xv_r[sc])
                    for s2 in range(SC // P):
                        sb = sc * (SC // P) + s2
                        ps2 = pav.tile([P, NH, P], F32, tag="pav", name="ps2")
                        for db in range(NDB):
                            nc.tensor.matmul(
                                ps2[:],
                                lhsT=xt[:, db, s2 * P:(s2 + 1) * P],
                                rhs=wv_sb[:, db, :],
                                start=(db == 0),
                                stop=(db == NDB - 1),
                            )
                        nc.vector.tensor_tensor(vpB[sb][:, :, 0:P], ps2[:],
                                                bvb_sb[:], AluOp.add)
                        nc.sync.dma_start(vp_r[sb], vpB[sb][:, :, 0:P])

            # attention PSUM pools (opened after pav closes: stay <= 8 banks)
            scps = actx.enter_context(tc.tile_pool(name="scps", bufs=2,
                                                   space="PSUM"))
            pvps = actx.enter_context(tc.tile_pool(name="pvps", bufs=2,
                                                   space="PSUM"))
            trps = actx.enter_context(tc.tile_pool(name="trps", bufs=2,
                                                   space="PSUM"))

            # ---- q projection interleaved with windowed attention ----
            for sc in range(NSC):
                xt = xpool.tile([P, NDB, SC], CDT, tag="x", name="xt")
                nc.sync.dma_start(xt[:], xq_r[sc])
                ssl = slice(sc * SC, (sc + 1) * SC)
                for hb in range(NH):
                    ps = pa.tile([P, SC], F32, tag="pa", name="ps")
                    for db in range(NDB):
                        nc.tensor.matmul(
                            ps[:],
                            lhsT=wq_sb[:, db, hb * P:(hb + 1) * P],
                            rhs=xt[:, db, :],
                            start=(db == 0),
                            stop=(db == NDB - 1),
                        )
                    nc.vector.tensor_scalar_add(qpT[hb][:, ssl], ps[:],
                                                bq_sb[:, hb:hb + 1])

                # scores + exp + masks for key blocks unlocked by this chunk
                for h in range(NH):
                    for kb in _KB_BATCH[sc]:
                        nj = min(3, NSB - kb)
                        kr = kb % NRING
                        scp = scps.tile([P, 3 * P], F32, tag="sc", name="scp")
                        nc.tensor.matmul(
                            scp[:, :nj * P],
                            lhsT=kpT[h][:, kb * P:(kb + 1) * P],
                            rhs=qpT[h][:, kb * P:(kb + nj) * P],
                            start=True,
                            stop=True,
                        )
                        nc.scalar.activation(expT[h][:, kr, 0:nj * P],
                                             scp[:, :nj * P], ActFn.Exp,
                                             scale=float(SCALE))
                        # diag block: keep q >= k  (iota = qf - kp >= 0)
                        nc.gpsimd.affine_select(
                            out=expT[h][:, kr, 0:P],
                            in_=expT[h][:, kr, 0:P],
                            pattern=[[1, P]],
                            channel_multiplier=-1,
                            base=0,
                            compare_op=AluOp.is_ge,
                            fill=0.0,
                        )
                        if nj == 3:
                            # far block: keep q <= k  (iota = kp - qf >= 0)
                            nc.gpsimd.affine_select(
                                out=expT[h][:, kr, 2 * P:3 * P],
                                in_=expT[h][:, kr, 2 * P:3 * P],
                                pattern=[[-1, P]],
                                channel_multiplier=1,
                                base=0,
                                compare_op=AluOp.is_ge,
                                fill=0.0,
                            )

                # PV + normalize + transpose for query blocks now complete
                for h in range(NH):
                    for qb in _QB_BATCH[sc]:
                        kb0 = max(0, qb - 2)
                        pv = pvps.tile([P, P + 1], F32, tag="pv", name="pv")
                        for kb in range(kb0, qb + 1):
                            rel = qb - kb
                            nc.tensor.matmul(
                                pv[:],
                                lhsT=expT[h][:, kb % NRING,
                                             rel * P:(rel + 1) * P],
                                rhs=vpB[kb][:, h, :],
                                start=(kb == kb0),
                                stop=(kb == qb),
                            )
                        rinv = awk.tile([P, 1], F32, tag="rinv", name="rinv")
                        nc.vector.reciprocal(rinv[:], pv[:, P:P + 1])
                        csb = awk.tile([P, P], CDT, tag="csb", name="csb")
                        nc.vector.tensor_scalar_mul(csb[:], pv[:, 0:P],
                                                    rinv[:])
                        tp = trps.tile([P, P], CDT, tag="tp", name="tp")
                        nc.tensor.transpose(tp[:], csb[:], ident[:])
                        nc.scalar.copy(ctxT[h][:, qb * P:(qb + 1) * P], tp[:])

        # ---- phase C: partial out-projection pout = ctx @ Wo_g ----
        with ExitStack() as cctx:
            cpool = cctx.enter_context(tc.tile_pool(name="cpool", bufs=2))
            psC = cctx.enter_context(tc.tile_pool(name="psC", bufs=2,
                                                  space="PSUM"))
            pout_r = pout_o.ap().rearrange("(sb p) n -> sb p n", p=P)
            for sb in range(NSB):
                po = cpool.tile([P, D], CDT, tag="po", name="po")
                for ec in range(D // 512):
                    esl = slice(ec * 512, (ec + 1) * 512)
                    psq = psC.tile([P, 512], F32, tag="psq", name="psq")
                    for h in range(NH):
                        nc.tensor.matmul(
                            psq[:],
                            lhsT=ctxT[h][:, sb * P:(sb + 1) * P],
                            rhs=wo_sb[:, h, esl],
                            start=(h == 0),
                            stop=(h == NH - 1),
                        )
                    nc.scalar.copy(po[:, esl], psq[:])
                    nc.sync.dma_start(pout_r[sb][:, esl], po[:, esl])

    nc.compile()
    return nc


def kernel(q, k, v, Wq, bq, Wk, bk, Wv, bv, Wo, bo):
    global LAST_RESULTS
    import ml_dtypes

    cdt = ml_dtypes.bfloat16
    q = np.asarray(q, np.float32)
    k = np.asarray(k, np.float32)
    v = np.asarray(v, np.float32)
    Wq = np.asarray(Wq, np.float32)
    Wk = np.asarray(Wk, np.float32)
    Wv = np.asarray(Wv, np.float32)
    Wo = np.asarray(Wo, np.float32)
    bq = np.asarray(bq, np.float32)
    bk = np.asarray(bk, np.float32)
    bv = np.asarray(bv, np.float32)
    bo = np.asarray(bo, np.float32)

    if "nc" not in _CACHE:
        _CACHE["nc"] = _build_nc()
    nc = _CACHE["nc"]
    from concourse.bass_utils import run_bass_kernel_spmd

    def pack_x(x):  # [S, D] -> [(c p), (do s)] with x^T chunked along seq
        a = x.T.reshape(NDB, P, NSC, SC)
        return np.ascontiguousarray(
            a.transpose(2, 1, 0, 3).reshape(NSC * P, NDB * SC)).astype(cdt)

    def pack_w(W, gsl):  # [D, D] cols gsl -> [p, (do n)]
        a = W[:, gsl].reshape(NDB, P, GD)
        return np.ascontiguousarray(
            a.transpose(1, 0, 2).reshape(P, NDB * GD)).astype(cdt)

    def pack_wo(W, gsl):  # rows gsl -> [p, (h n)]
        a = W[gsl, :].reshape(NH, P, D)
        return np.ascontiguousarray(
            a.transpose(1, 0, 2).reshape(P, NH * D)).astype(cdt)

    xs = {}
    for b in range(B):
        xs[("q", b)] = pack_x(q[b])
        xs[("k", b)] = pack_x(k[b])
        xs[("v", b)] = pack_x(v[b])

    in_maps = []
    for core in range(8):
        b, g = divmod(core, 4)
        gsl = slice(g * GD, (g + 1) * GD)
        in_maps.append({
            "xq_c": xs[("q", b)],
            "xk_c": xs[("k", b)],
            "xv_c": xs[("v", b)],
            "wq_c": pack_w(Wq, gsl),
            "wk_c": pack_w(Wk, gsl),
            "wv_c": pack_w(Wv, gsl),
            "wo_c": pack_wo(Wo, gsl),
            "bq_c": np.ascontiguousarray(bq[gsl].reshape(NH, P).T),
            "bk_c": np.ascontiguousarray(bk[gsl].reshape(NH, P).T),
            "bvb": np.ascontiguousarray(
                np.broadcast_to(bv[gsl], (P, GD))).astype(np.float32),
        })

    trace = os.environ.get("KERNEL_TRACE", "0") == "1"
    res = run_bass_kernel_spmd(nc, in_maps, core_ids=list(range(8)),
                               trace=trace)
    LAST_RESULTS = res

    out = np.zeros((B, S, D), np.float64)
    kp = np.empty((B, S, D), np.float32)
    vp = np.empty((B, S, D), np.float32)
    for core in range(8):
        b, g = divmod(core, 4)
        gsl = slice(g * GD, (g + 1) * GD)
        r = res.results[core]
        kp[b][:, gsl] = r["kp"].astype(np.float32).T
        vp[b][:, gsl] = r["vp"].astype(np.float32)
        out[b] += r["pout"].astype(np.float64)
    out = (out + bo.astype(np.float64)).astype(np.float32)
    return out, kp, vp
